# revision 11
# baseline (speedup 1.0000x reference)
"""Trainium2 Bass kernel for nn_MoE_32332513804634.

MoE: 16 routed experts (top-6, softmax-then-bias routing) + dense shared
expert, T=4096 tokens, D=2048, H=1408, HS=2816, fp32.

Strategy (8 NeuronCores, SPMD):
  - Host computes the gate (cheap: 0.27 GFLOP) and per-expert token lists.
  - Uniform slot profile: a small search finds per-core slot sizes
    (multiples of 128, identical across cores -- SPMD needs one program)
    that cover the 16 expert token counts with minimal padding (~4% vs
    the 17% of fixed 2048/512 caps). Each slot is bound per-core to one
    (expert, offset, len) piece; every core runs exactly the same
    instruction stream over sum(profile) routed tokens.
  - Within a slot: x is resident in SBUF, weights stream hm-outer so each
    weight byte is DMA'd once per slot. Layer-2 output is produced
    transposed [D, tokens] (tokens stay on the moving dim -- cost scales
    exactly with tokens, no 128-alignment of token counts needed).
  - Shared expert is token-parallel: each core runs its 512-token slice
    through the full 2816 hidden dim as one extra slot (no padding).
  - bf16 matmul operands + bf16 outputs; fp32 PSUM accumulation. Combine
    weights, b2/bs2 biases, scatter and the 8-way shared sum happen on
    the host in fp32.
"""

import sys
import numpy as np

sys.path.insert(0, "/opt/trn_rl_repo")

import concourse.bass as bass  # noqa: E402
import concourse.tile as tile  # noqa: E402
from concourse import bacc, mybir  # noqa: E402
from concourse.bass_utils import run_bass_kernel_spmd  # noqa: E402

T = 4096
D = 2048
H = 1408
E = 16
TOP_K = 6
HS = 2816
N_CORES = 8
KO = D // 128           # 16
HM = H // 128           # 11
HMS = HS // 128         # 22
SH_TOK = T // N_CORES   # 512 shared-expert tokens per core
MAX_SLOT = 1024
F32 = mybir.dt.float32
BF16 = mybir.dt.bfloat16
MM_DT = BF16

_PROGRAM_CACHE: dict = {}

import ml_dtypes  # noqa: E402

BF16_NP = ml_dtypes.bfloat16


def _host_gate(xf, gate_w, gate_b):
    """Numpy replica of the reference gate. Returns cw [T, E] dense combine
    weights and per-expert token lists (ascending)."""
    scores = xf @ gate_w.T
    m = scores.max(axis=-1, keepdims=True)
    p = np.exp(scores - m, dtype=np.float32)
    probs = p / p.sum(axis=-1, keepdims=True)
    biased = probs + gate_b
    idx = np.argpartition(biased, E - TOP_K, axis=1)[:, E - TOP_K:]
    mask = np.zeros((xf.shape[0], E), dtype=bool)
    mask[np.arange(xf.shape[0])[:, None], idx] = True
    cw = np.where(mask, probs, 0.0).astype(np.float32)
    toks = [np.flatnonzero(mask[:, e]).astype(np.int64) for e in range(E)]
    return cw, toks


def _cover(cs, slots):
    """Greedy cover: for each count (desc) take largest remaining slots until
    covered, then shrink the last taken to the smallest adequate one."""
    if not cs:
        return []
    c = cs[0][1]
    take = []
    s = 0
    for sz in slots:
        if s >= c:
            break
        take.append(sz)
        s += sz
    if s < c:
        return None
    if take:
        need = c - (s - take[-1])
        rest = list(slots)
        for t in take[:-1]:
            rest.remove(t)
        cands = [sz for sz in set(rest) if sz >= need]
        if cands:
            take = take[:-1] + [min(cands)]
    rem = list(slots)
    for t in take:
        rem.remove(t)
    rem.sort(reverse=True)
    sub = _cover(cs[1:], rem)
    if sub is None:
        return None
    return [take] + sub


def _plan_profile(counts):
    """Find per-core slot sizes (desc, multiples of 128, <= MAX_SLOT) common
    to all cores that cover the expert counts with minimal total padding.

    Returns (profile, assignment) where assignment[core][j] is
    (expert, start, n) or None."""
    order = np.argsort(counts)[::-1]
    cs = [(int(e), int(counts[e])) for e in order if counts[e] > 0]

    def parts(n, maxp, maxsz):
        if n == 0:
            yield ()
            return
        if maxp == 0:
            return
        for sz in range(min(n, maxsz), 0, -1):
            for rest in parts(n - sz, maxp - 1, sz):
                yield (sz,) + rest

    found = None
    for total_u in range(-(-sum(counts) // (128 * N_CORES)), 64):
        for prof_u in parts(total_u, 7, MAX_SLOT // 128):
            prof = tuple(s * 128 for s in prof_u)
            slots = sorted(list(prof) * N_CORES, reverse=True)
            r = _cover(cs, slots)
            if r is not None:
                found = (prof, r)
                break
        if found:
            break
    assert found is not None
    prof, groups = found
    # interleave big/small slots so every small (weight-DMA-heavy) slot
    # executes right after a big slot whose compute prefetches its weights
    ps = sorted(prof, reverse=True)
    half = (len(ps) + 1) // 2
    inter = []
    for a, b in zip(ps[:half], ps[half:] + [None]):
        inter.append(a)
        if b is not None:
            inter.append(b)
    prof = tuple(inter)
    # slot instances: per size, list of (core, j)
    inst = {}
    for j, sz in enumerate(prof):
        for c in range(N_CORES):
            inst.setdefault(sz, []).append((c, j))
    assignment = [[None] * len(prof) for _ in range(N_CORES)]
    for (e, cnt), gslots in zip(cs, groups):
        off = 0
        for sz in sorted(gslots, reverse=True):
            c, j = inst[sz].pop()
            n = min(sz, cnt - off)
            if n > 0:
                assignment[c][j] = (e, off, n)
            off += n
    return prof, assignment


def _build_program(profile):
    """SPMD Bass program: len(profile) routed slots + 1 shared slot."""
    nc = bacc.Bacc("TRN2", debug=False, num_devices=N_CORES)

    n_slots = len(profile)
    C = sum(profile) + SH_TOK  # total token columns per core
    ms = max(max(profile), SH_TOK)

    ins = {}

    def din(name, shape, dt=MM_DT):
        ins[name] = nc.dram_tensor(name, list(shape), dt, kind="ExternalInput").ap()
        return ins[name]

    xg = din("xg", (128, KO, C))
    for j, s in enumerate(profile):
        din(f"w1_{j}", (HM, 128, KO, 128))
        din(f"w3_{j}", (HM, 128, KO, 128))
        din(f"w2_{j}", (KO, 128, HM, 128))
        din(f"b1_{j}", (128, HM), F32)
        din(f"b3_{j}", (128, HM), F32)
    din("ws1", (HMS, 128, KO, 128))
    din("ws3", (HMS, 128, KO, 128))
    din("ws2", (KO, 128, HMS, 128))
    din("bs1", (128, HMS), F32)
    din("bs3", (128, HMS), F32)
    oe = nc.dram_tensor("oe", [KO, 128, C], BF16, kind="ExternalOutput").ap()

    h_flat = max(HM * max(profile), HMS * SH_TOK)

    with tile.TileContext(nc) as tc:
        with (
            tc.tile_pool(name="xpool", bufs=4) as xpool,
            tc.tile_pool(name="hpool", bufs=1) as hpool,
            tc.tile_pool(name="wcol", bufs=11) as wcol,
            tc.tile_pool(name="w2pool", bufs=4) as w2pool,
            tc.tile_pool(name="tmp", bufs=2) as tmp,
            tc.tile_pool(name="opool", bufs=4) as opool,
            tc.tile_pool(name="cpool", bufs=1) as cpool,
            tc.tile_pool(name="pp", bufs=2, space="PSUM") as pp,
        ):
            # warm the PE (and its HAM clock gate) with throwaway matmuls
            # while the first slot's x/weight DMAs are in flight
            wsc1 = cpool.tile([128, 128], MM_DT, tag="wsc1")
            wsc2 = cpool.tile([128, 512], MM_DT, tag="wsc2")
            nc.vector.memset(wsc1[:], 0)
            nc.vector.memset(wsc2[:], 0)
            for _ in range(40):
                pw = pp.tile([128, 512], F32, tag="ps1")
                nc.tensor.matmul(pw[:], wsc1[:], wsc2[:], start=True,
                                 stop=True)
            def mlp_slot(c0, s, n_hm, w1_ap, w3_ap, w2_ap, b1_ap, b3_ap, tag):
                """out[:, c0:c0+s] = swiglu_mlp(x[:, c0:c0+s]) (no out bias)."""
                chunks = [(t0, min(512, s - t0)) for t0 in range(0, s, 512)]

                b1sb = cpool.tile([128, n_hm], F32, tag=f"b1{tag}")
                b3sb = cpool.tile([128, n_hm], F32, tag=f"b3{tag}")
                nc.sync.dma_start(b1sb[:], b1_ap)
                nc.sync.dma_start(b3sb[:], b3_ap)

                xts = []
                for (t0, n) in chunks:
                    xt = xpool.tile([128, KO, 512], MM_DT, tag="x")
                    # split across 4 descriptors so 4 DMA engines serve it
                    for q in range(4):
                        nc.sync.dma_start(xt[:, 4 * q:4 * q + 4, :n],
                                          xg[:, 4 * q:4 * q + 4,
                                             c0 + t0:c0 + t0 + n])
                    xts.append(xt)

                hsb = hpool.tile([128, h_flat], MM_DT, tag="h")
                # w2 for the first dms: emitted up front so the scalar queue
                # fires them during L1 (their ring waits are already met)
                w2ts = []
                for dm in range(KO):
                    if dm < 4:
                        w2t = w2pool.tile([128, HMS, 128], MM_DT, tag="w2c")
                        nc.scalar.dma_start(w2t[:, :n_hm, :], w2_ap[dm])
                        w2ts.append(w2t)
                # ---- layer 1: h = silu(x@W1+b1) * (x@W3+b3), hm-outer ----
                for hm in range(n_hm):
                    w1t = wcol.tile([128, KO, 128], MM_DT, tag="w1c")
                    nc.sync.dma_start(w1t[:], w1_ap[hm])
                    w3t = wcol.tile([128, KO, 128], MM_DT, tag="w3c")
                    nc.sync.dma_start(w3t[:], w3_ap[hm])
                    for ci, (t0, n) in enumerate(chunks):
                        xt = xts[ci]
                        ps1 = pp.tile([128, 512], F32, tag="ps1")
                        for ko in range(KO):
                            nc.tensor.matmul(ps1[:, :n], w1t[:, ko, :],
                                             xt[:, ko, :n],
                                             start=(ko == 0),
                                             stop=(ko == KO - 1))
                        ps3 = pp.tile([128, 512], F32, tag="ps3")
                        for ko in range(KO):
                            nc.tensor.matmul(ps3[:, :n], w3t[:, ko, :],
                                             xt[:, ko, :n],
                                             start=(ko == 0),
                                             stop=(ko == KO - 1))
                        t1 = tmp.tile([128, 512], BF16, tag="t1")
                        nc.scalar.activation(t1[:, :n], ps1[:, :n],
                                             mybir.ActivationFunctionType.Silu,
                                             bias=b1sb[:, hm:hm + 1])
                        t3 = tmp.tile([128, 512], BF16, tag="t3")
                        nc.vector.tensor_scalar_add(t3[:, :n], ps3[:, :n],
                                                    b3sb[:, hm:hm + 1])
                        ho = hm * s + t0
                        nc.vector.tensor_mul(hsb[:, ho:ho + n],
                                             t1[:, :n], t3[:, :n])
                # ---- layer 2: out[d, t] = sum_h W2[d, h] h[h, t] ----
                for dm in range(KO):
                    w2t = w2ts[dm]
                    for (t0, n) in chunks:
                        ps2 = pp.tile([128, 512], F32, tag="ps2", bufs=3)
                        for hk in range(n_hm):
                            ho = hk * s + t0
                            nc.tensor.matmul(ps2[:, :n], w2t[:, hk, :],
                                             hsb[:, ho:ho + n],
                                             start=(hk == 0),
                                             stop=(hk == n_hm - 1))
                        ob = opool.tile([128, 512], BF16, tag="ob")
                        nc.vector.tensor_copy(ob[:, :n], ps2[:, :n])
                        nc.scalar.dma_start(
                            oe[dm][:, c0 + t0:c0 + t0 + n], ob[:, :n])
                    if dm + 4 < KO:
                        w2t = w2pool.tile([128, HMS, 128], MM_DT, tag="w2c")
                        nc.scalar.dma_start(w2t[:, :n_hm, :], w2_ap[dm + 4])
                        w2ts.append(w2t)

            c0 = 0
            for j, s in enumerate(profile):
                mlp_slot(c0, s, HM, ins[f"w1_{j}"], ins[f"w3_{j}"],
                         ins[f"w2_{j}"], ins[f"b1_{j}"], ins[f"b3_{j}"],
                         f"e{j}")
                c0 += s
            mlp_slot(c0, SH_TOK, HMS, ins["ws1"], ins["ws3"], ins["ws2"],
                     ins["bs1"], ins["bs3"], "sh")

    nc.compile()
    return nc


def _pack_w13(w):
    """[H', D] -> [H'/128, 128(d%128), KO, 128(h%128)] bf16 (lhsT tiles)."""
    hm = w.shape[0] // 128
    return np.ascontiguousarray(
        w.reshape(hm, 128, KO, 128).transpose(0, 3, 2, 1)).astype(BF16_NP)


def _pack_w2(w):
    """[D, H'] -> [KO, 128(h%128), H'/128, 128(d%128)] bf16 (lhsT tiles)."""
    hm = w.shape[1] // 128
    return np.ascontiguousarray(
        w.reshape(KO, 128, hm, 128).transpose(0, 3, 2, 1)).astype(BF16_NP)


def kernel(x, gate_w, gate_b, w1, b1, w2, b2, w3, b3,
           ws1, bs1, ws2, bs2, ws3, bs3):
    x = np.asarray(x, np.float32)
    xf = np.ascontiguousarray(x.reshape(-1, D))
    gate_w = np.asarray(gate_w, np.float32)
    gate_b = np.asarray(gate_b, np.float32)
    w1 = np.asarray(w1, np.float32)
    b1 = np.asarray(b1, np.float32)
    w2 = np.asarray(w2, np.float32)
    b2 = np.asarray(b2, np.float32)
    w3 = np.asarray(w3, np.float32)
    b3 = np.asarray(b3, np.float32)
    ws1 = np.asarray(ws1, np.float32)
    bs1 = np.asarray(bs1, np.float32)
    ws2 = np.asarray(ws2, np.float32)
    bs2 = np.asarray(bs2, np.float32)
    ws3 = np.asarray(ws3, np.float32)
    bs3 = np.asarray(bs3, np.float32)

    cw, toks = _host_gate(xf, gate_w, gate_b)
    counts = np.array([len(t) for t in toks])
    profile, assignment = _plan_profile(counts)

    if profile not in _PROGRAM_CACHE:
        _PROGRAM_CACHE[profile] = _build_program(profile)
    nc = _PROGRAM_CACHE[profile]

    C = sum(profile) + SH_TOK
    xT = xf.T  # [D, T] view

    # per-expert packed weights, shared across cores/slots
    need = sorted({p[0] for slots in assignment for p in slots if p is not None})
    w1p = {e: _pack_w13(w1[e]) for e in need}
    w3p = {e: _pack_w13(w3[e]) for e in need}
    w2p = {e: _pack_w2(w2[e]) for e in need}
    b1p = {e: np.ascontiguousarray(b1[e].reshape(HM, 128).T) for e in need}
    b3p = {e: np.ascontiguousarray(b3[e].reshape(HM, 128).T) for e in need}
    zb = np.zeros((128, HM), np.float32)

    ws1p = _pack_w13(ws1)
    ws3p = _pack_w13(ws3)
    ws2p = _pack_w2(ws2)
    bs1p = np.ascontiguousarray(bs1.reshape(HMS, 128).T)
    bs3p = np.ascontiguousarray(bs3.reshape(HMS, 128).T)

    in_maps = []
    for c in range(N_CORES):
        m = {}
        xcols = np.zeros((D, C), np.float32)
        c0 = 0
        for j, s in enumerate(profile):
            piece = assignment[c][j]
            if piece is None:
                e0 = need[0]
                m[f"w1_{j}"] = w1p[e0]
                m[f"w3_{j}"] = w3p[e0]
                m[f"w2_{j}"] = w2p[e0]
                m[f"b1_{j}"] = zb
                m[f"b3_{j}"] = zb
            else:
                e, s0, n = piece
                tk = toks[e][s0:s0 + n]
                xcols[:, c0:c0 + n] = xT[:, tk]
                m[f"w1_{j}"] = w1p[e]
                m[f"w3_{j}"] = w3p[e]
                m[f"w2_{j}"] = w2p[e]
                m[f"b1_{j}"] = b1p[e]
                m[f"b3_{j}"] = b3p[e]
            c0 += s
        xcols[:, c0:c0 + SH_TOK] = xT[:, c * SH_TOK:(c + 1) * SH_TOK]
        m["xg"] = np.ascontiguousarray(
            xcols.reshape(KO, 128, C).transpose(1, 0, 2)).astype(BF16_NP)
        m["ws1"] = ws1p
        m["ws3"] = ws3p
        m["ws2"] = ws2p
        m["bs1"] = bs1p
        m["bs3"] = bs3p
        in_maps.append(m)

    res = run_bass_kernel_spmd(nc, in_maps, list(range(N_CORES)))

    # host combine: scatter slot outputs, apply combine weights + b2, add
    # shared partials + bs2
    y = np.zeros((T, D), np.float32)
    for c in range(N_CORES):
        out = res.results[c]["oe"].astype(np.float32).reshape(D, C)
        c0 = 0
        for j, s in enumerate(profile):
            piece = assignment[c][j]
            if piece is not None:
                e, s0, n = piece
                tk = toks[e][s0:s0 + n]
                cwe = cw[tk, e][:, None]
                y[tk] += cwe * out[:, c0:c0 + n].T
                y[tk] += cwe * b2[e][None, :]
            c0 += s
        y[c * SH_TOK:(c + 1) * SH_TOK] += out[:, c0:c0 + SH_TOK].T
    y += bs2[None, :]
    return y.reshape(x.shape).astype(np.float32)


# revision 20
# speedup vs baseline: 1.0071x; 1.0071x over previous
"""Trainium2 Bass kernel for nn_MoE_32332513804634.

MoE: 16 routed experts (top-6, softmax-then-bias routing) + dense shared
expert, T=4096 tokens, D=2048, H=1408, HS=2816, fp32.

Strategy (8 NeuronCores, SPMD):
  - Host computes the gate (cheap: 0.27 GFLOP) and per-expert token lists.
  - Uniform slot profile: a small search finds per-core slot sizes
    (multiples of 128, identical across cores -- SPMD needs one program)
    that cover the 16 expert token counts with minimal padding (~4% vs
    the 17% of fixed 2048/512 caps). Each slot is bound per-core to one
    (expert, offset, len) piece; every core runs exactly the same
    instruction stream over sum(profile) routed tokens.
  - Within a slot: x is resident in SBUF, weights stream hm-outer so each
    weight byte is DMA'd once per slot. Layer-2 output is produced
    transposed [D, tokens] (tokens stay on the moving dim -- cost scales
    exactly with tokens, no 128-alignment of token counts needed).
  - Shared expert is token-parallel: each core runs its 512-token slice
    through the full 2816 hidden dim as one extra slot (no padding).
  - bf16 matmul operands + bf16 outputs; fp32 PSUM accumulation. Combine
    weights, b2/bs2 biases, scatter and the 8-way shared sum happen on
    the host in fp32.
"""

import sys
import numpy as np

sys.path.insert(0, "/opt/trn_rl_repo")

import concourse.bass as bass  # noqa: E402
import concourse.tile as tile  # noqa: E402
from concourse import bacc, mybir  # noqa: E402
from concourse.bass_utils import run_bass_kernel_spmd  # noqa: E402

T = 4096
D = 2048
H = 1408
E = 16
TOP_K = 6
HS = 2816
N_CORES = 8
KO = D // 128           # 16
HM = H // 128           # 11
HMS = HS // 128         # 22
SH_TOK = T // N_CORES   # 512 shared-expert tokens per core
MAX_SLOT = 1024
F32 = mybir.dt.float32
BF16 = mybir.dt.bfloat16
MM_DT = BF16

_PROGRAM_CACHE: dict = {}

import ml_dtypes  # noqa: E402

BF16_NP = ml_dtypes.bfloat16


def _host_gate(xf, gate_w, gate_b):
    """Numpy replica of the reference gate. Returns cw [T, E] dense combine
    weights and per-expert token lists (ascending)."""
    scores = xf @ gate_w.T
    m = scores.max(axis=-1, keepdims=True)
    p = np.exp(scores - m, dtype=np.float32)
    probs = p / p.sum(axis=-1, keepdims=True)
    biased = probs + gate_b
    idx = np.argpartition(biased, E - TOP_K, axis=1)[:, E - TOP_K:]
    mask = np.zeros((xf.shape[0], E), dtype=bool)
    mask[np.arange(xf.shape[0])[:, None], idx] = True
    cw = np.where(mask, probs, 0.0).astype(np.float32)
    toks = [np.flatnonzero(mask[:, e]).astype(np.int64) for e in range(E)]
    return cw, toks


def _cover(cs, slots):
    """Greedy cover: for each count (desc) take largest remaining slots until
    covered, then shrink the last taken to the smallest adequate one."""
    if not cs:
        return []
    c = cs[0][1]
    take = []
    s = 0
    for sz in slots:
        if s >= c:
            break
        take.append(sz)
        s += sz
    if s < c:
        return None
    if take:
        need = c - (s - take[-1])
        rest = list(slots)
        for t in take[:-1]:
            rest.remove(t)
        cands = [sz for sz in set(rest) if sz >= need]
        if cands:
            take = take[:-1] + [min(cands)]
    rem = list(slots)
    for t in take:
        rem.remove(t)
    rem.sort(reverse=True)
    sub = _cover(cs[1:], rem)
    if sub is None:
        return None
    return [take] + sub


def _plan_profile(counts):
    """Find per-core slot sizes (desc, multiples of 128, <= MAX_SLOT) common
    to all cores that cover the expert counts with minimal total padding.

    Returns (profile, assignment) where assignment[core][j] is
    (expert, start, n) or None."""
    order = np.argsort(counts)[::-1]
    cs = [(int(e), int(counts[e])) for e in order if counts[e] > 0]

    def parts(n, maxp, maxsz):
        if n == 0:
            yield ()
            return
        if maxp == 0:
            return
        for sz in range(min(n, maxsz), 0, -1):
            for rest in parts(n - sz, maxp - 1, sz):
                yield (sz,) + rest

    found = None
    for total_u in range(-(-sum(counts) // (128 * N_CORES)), 64):
        for prof_u in parts(total_u, 7, MAX_SLOT // 128):
            prof = tuple(s * 128 for s in prof_u)
            slots = sorted(list(prof) * N_CORES, reverse=True)
            r = _cover(cs, slots)
            if r is not None:
                found = (prof, r)
                break
        if found:
            break
    assert found is not None
    prof, groups = found
    # order slots big-then-its-small: each weight-DMA-heavy small slot runs
    # right after a big slot whose compute window prefetches its weights
    ps = sorted(prof, reverse=True)
    inter = []
    for i in range((len(ps) + 1) // 2):
        inter.append(ps[i])
        j = len(ps) - 1 - i
        if j > i:
            inter.append(ps[j])
    prof = tuple(inter)
    # slot instances: per size, list of (core, j)
    inst = {}
    for j, sz in enumerate(prof):
        for c in range(N_CORES):
            inst.setdefault(sz, []).append((c, j))
    assignment = [[None] * len(prof) for _ in range(N_CORES)]
    for (e, cnt), gslots in zip(cs, groups):
        off = 0
        for sz in sorted(gslots, reverse=True):
            c, j = inst[sz].pop()
            n = min(sz, cnt - off)
            if n > 0:
                assignment[c][j] = (e, off, n)
            off += n
    return prof, assignment


def _build_program(profile):
    """SPMD Bass program: len(profile) routed slots + 1 shared slot."""
    nc = bacc.Bacc("TRN2", debug=False, num_devices=N_CORES)

    sizes = list(profile) + [SH_TOK]
    nch = [-(-s // 512) for s in sizes]    # chunks per slot
    NCH = sum(nch)                          # total 512-wide chunk blocks
    C = NCH * 512                           # col space (chunk grid)

    ins = {}

    def din(name, shape, dt=MM_DT):
        ins[name] = nc.dram_tensor(name, list(shape), dt, kind="ExternalInput").ap()
        return ins[name]

    # x gathered tokens, chunk-major: 16 KB contiguous per partition row
    xg = din("xg", (NCH, 128, KO * 512))
    for j, s in enumerate(profile):
        din(f"w1_{j}", (HM, 128, KO, 128))
        din(f"w3_{j}", (HM, 128, KO, 128))
        din(f"w2_{j}", (KO, 128, HM, 128))
        din(f"b1_{j}", (128, HM), F32)
        din(f"b3_{j}", (128, HM), F32)
    din("ws1", (HMS, 128, KO, 128))
    din("ws3", (HMS, 128, KO, 128))
    din("ws2", (KO, 128, HMS, 128))
    din("bs1", (128, HMS), F32)
    din("bs3", (128, HMS), F32)
    oe = nc.dram_tensor("oe", [KO, 128, C], BF16, kind="ExternalOutput").ap()

    h_flat = max(HM * max(profile), HMS * SH_TOK)

    with tile.TileContext(nc) as tc:
        with (
            tc.tile_pool(name="xpool", bufs=3) as xpool,
            tc.tile_pool(name="hpool", bufs=1) as hpool,
            tc.tile_pool(name="wcol", bufs=11) as wcol,
            tc.tile_pool(name="w2pool", bufs=4) as w2pool,
            tc.tile_pool(name="tmp", bufs=2) as tmp,
            tc.tile_pool(name="opool", bufs=3) as opool,
            tc.tile_pool(name="cpool", bufs=1) as cpool,
            tc.tile_pool(name="pp", bufs=2, space="PSUM") as pp,
        ):
            # warm the PE (and its HAM clock gate) with throwaway matmuls
            # while the first slot's x/weight DMAs are in flight
            wsc1 = cpool.tile([128, 128], MM_DT, tag="wsc1")
            wsc2 = cpool.tile([128, 512], MM_DT, tag="wsc2")
            nc.vector.memset(wsc1[:], 0)
            nc.vector.memset(wsc2[:], 0)
            for _ in range(40):
                pw = pp.tile([128, 512], F32, tag="ps1")
                nc.tensor.matmul(pw[:], wsc1[:], wsc2[:], start=True,
                                 stop=True)
            def mlp_slot(cb, s, n_hm, w1_ap, w3_ap, w2_ap, b1_ap, b3_ap, tag):
                """out chunk-blocks [cb, cb+nch) = swiglu_mlp(x blocks)."""
                chunks = [(t0, min(512, s - t0)) for t0 in range(0, s, 512)]
                ncs = len(chunks)

                b1sb = cpool.tile([128, n_hm], F32, tag=f"b1{tag}")
                b3sb = cpool.tile([128, n_hm], F32, tag=f"b3{tag}")
                nc.scalar.dma_start(b1sb[:], b1_ap)
                nc.scalar.dma_start(b3sb[:], b3_ap)

                xts = []
                for ci, (t0, n) in enumerate(chunks):
                    xt = xpool.tile([128, KO, 512], MM_DT, tag="x")
                    nc.scalar.dma_start(
                        xt[:], xg[cb + ci].rearrange("p (a b) -> p a b", a=KO))
                    xts.append(xt)

                hsb = hpool.tile([128, h_flat], MM_DT, tag="h")
                # w2 for the first dms: emitted up front so the scalar queue
                # fires them during L1 (their ring waits are already met)
                w2ts = []
                for dm in range(4):
                    w2t = w2pool.tile([128, HMS, 128], MM_DT, tag="w2c")
                    nc.scalar.dma_start(w2t[:, :n_hm, :], w2_ap[dm])
                    w2ts.append(w2t)
                # ---- layer 1: h = silu(x@W1+b1) * (x@W3+b3), hm-outer ----
                for hm in range(n_hm):
                    w1t = wcol.tile([128, KO, 128], MM_DT, tag="w1c")
                    nc.sync.dma_start(w1t[:], w1_ap[hm])
                    w3t = wcol.tile([128, KO, 128], MM_DT, tag="w3c")
                    nc.sync.dma_start(w3t[:], w3_ap[hm])
                    for ci, (t0, n) in enumerate(chunks):
                        xt = xts[ci]
                        ps1 = pp.tile([128, 512], F32, tag="ps1")
                        for ko in range(KO):
                            nc.tensor.matmul(ps1[:, :n], w1t[:, ko, :],
                                             xt[:, ko, :n],
                                             start=(ko == 0),
                                             stop=(ko == KO - 1))
                        ps3 = pp.tile([128, 512], F32, tag="ps3")
                        for ko in range(KO):
                            nc.tensor.matmul(ps3[:, :n], w3t[:, ko, :],
                                             xt[:, ko, :n],
                                             start=(ko == 0),
                                             stop=(ko == KO - 1))
                        t1 = tmp.tile([128, 512], BF16, tag="t1")
                        nc.scalar.activation(t1[:, :n], ps1[:, :n],
                                             mybir.ActivationFunctionType.Silu,
                                             bias=b1sb[:, hm:hm + 1])
                        t3 = tmp.tile([128, 512], BF16, tag="t3")
                        nc.vector.tensor_scalar_add(t3[:, :n], ps3[:, :n],
                                                    b3sb[:, hm:hm + 1])
                        ho = hm * s + t0
                        nc.vector.tensor_mul(hsb[:, ho:ho + n],
                                             t1[:, :n], t3[:, :n])
                # ---- layer 2: out[d, t] = sum_h W2[d, h] h[h, t] ----
                for dm in range(KO):
                    w2t = w2ts[dm]
                    ob = opool.tile([128, 1024], BF16, tag="ob")
                    for ci, (t0, n) in enumerate(chunks):
                        ps2 = pp.tile([128, 512], F32, tag="ps2", bufs=3)
                        for hk in range(n_hm):
                            ho = hk * s + t0
                            nc.tensor.matmul(ps2[:, :n], w2t[:, hk, :],
                                             hsb[:, ho:ho + n],
                                             start=(hk == 0),
                                             stop=(hk == n_hm - 1))
                        nc.vector.tensor_copy(ob[:, t0:t0 + n], ps2[:, :n])
                    # single store per dm: 2-chunk slots get 2 KB rows
                    nc.scalar.dma_start(
                        oe[dm][:, cb * 512:(cb + ncs) * 512],
                        ob[:, :ncs * 512])
                    if dm + 4 < KO:
                        w2t = w2pool.tile([128, HMS, 128], MM_DT, tag="w2c")
                        nc.scalar.dma_start(w2t[:, :n_hm, :], w2_ap[dm + 4])
                        w2ts.append(w2t)

            cb = 0
            for j, s in enumerate(profile):
                mlp_slot(cb, s, HM, ins[f"w1_{j}"], ins[f"w3_{j}"],
                         ins[f"w2_{j}"], ins[f"b1_{j}"], ins[f"b3_{j}"],
                         f"e{j}")
                cb += -(-s // 512)
            mlp_slot(cb, SH_TOK, HMS, ins["ws1"], ins["ws3"], ins["ws2"],
                     ins["bs1"], ins["bs3"], "sh")

    nc.compile()
    return nc


def _pack_w13(w):
    """[H', D] -> [H'/128, 128(d%128), KO, 128(h%128)] bf16 (lhsT tiles)."""
    hm = w.shape[0] // 128
    return np.ascontiguousarray(
        w.reshape(hm, 128, KO, 128).transpose(0, 3, 2, 1)).astype(BF16_NP)


def _pack_w2(w):
    """[D, H'] -> [KO, 128(h%128), H'/128, 128(d%128)] bf16 (lhsT tiles)."""
    hm = w.shape[1] // 128
    return np.ascontiguousarray(
        w.reshape(KO, 128, hm, 128).transpose(0, 3, 2, 1)).astype(BF16_NP)


def kernel(x, gate_w, gate_b, w1, b1, w2, b2, w3, b3,
           ws1, bs1, ws2, bs2, ws3, bs3):
    x = np.asarray(x, np.float32)
    xf = np.ascontiguousarray(x.reshape(-1, D))
    gate_w = np.asarray(gate_w, np.float32)
    gate_b = np.asarray(gate_b, np.float32)
    w1 = np.asarray(w1, np.float32)
    b1 = np.asarray(b1, np.float32)
    w2 = np.asarray(w2, np.float32)
    b2 = np.asarray(b2, np.float32)
    w3 = np.asarray(w3, np.float32)
    b3 = np.asarray(b3, np.float32)
    ws1 = np.asarray(ws1, np.float32)
    bs1 = np.asarray(bs1, np.float32)
    ws2 = np.asarray(ws2, np.float32)
    bs2 = np.asarray(bs2, np.float32)
    ws3 = np.asarray(ws3, np.float32)
    bs3 = np.asarray(bs3, np.float32)

    cw, toks = _host_gate(xf, gate_w, gate_b)
    counts = np.array([len(t) for t in toks])
    profile, assignment = _plan_profile(counts)

    if profile not in _PROGRAM_CACHE:
        _PROGRAM_CACHE[profile] = _build_program(profile)
    nc = _PROGRAM_CACHE[profile]

    # chunk-grid column bases: each slot occupies ceil(s/512) 512-wide blocks
    sizes = list(profile) + [SH_TOK]
    cbase = []
    acc = 0
    for s in sizes:
        cbase.append(acc * 512)
        acc += -(-s // 512)
    NCH = acc
    C = NCH * 512
    xT = xf.T  # [D, T] view

    # per-expert packed weights, shared across cores/slots
    need = sorted({p[0] for slots in assignment for p in slots if p is not None})
    w1p = {e: _pack_w13(w1[e]) for e in need}
    w3p = {e: _pack_w13(w3[e]) for e in need}
    w2p = {e: _pack_w2(w2[e]) for e in need}
    b1p = {e: np.ascontiguousarray(b1[e].reshape(HM, 128).T) for e in need}
    b3p = {e: np.ascontiguousarray(b3[e].reshape(HM, 128).T) for e in need}
    zb = np.zeros((128, HM), np.float32)

    ws1p = _pack_w13(ws1)
    ws3p = _pack_w13(ws3)
    ws2p = _pack_w2(ws2)
    bs1p = np.ascontiguousarray(bs1.reshape(HMS, 128).T)
    bs3p = np.ascontiguousarray(bs3.reshape(HMS, 128).T)

    in_maps = []
    for c in range(N_CORES):
        m = {}
        xcols = np.zeros((D, C), np.float32)
        for j, s in enumerate(profile):
            piece = assignment[c][j]
            if piece is None:
                e0 = need[0]
                m[f"w1_{j}"] = w1p[e0]
                m[f"w3_{j}"] = w3p[e0]
                m[f"w2_{j}"] = w2p[e0]
                m[f"b1_{j}"] = zb
                m[f"b3_{j}"] = zb
            else:
                e, s0, n = piece
                tk = toks[e][s0:s0 + n]
                xcols[:, cbase[j]:cbase[j] + n] = xT[:, tk]
                m[f"w1_{j}"] = w1p[e]
                m[f"w3_{j}"] = w3p[e]
                m[f"w2_{j}"] = w2p[e]
                m[f"b1_{j}"] = b1p[e]
                m[f"b3_{j}"] = b3p[e]
        xcols[:, cbase[-1]:cbase[-1] + SH_TOK] = \
            xT[:, c * SH_TOK:(c + 1) * SH_TOK]
        m["xg"] = np.ascontiguousarray(
            xcols.reshape(KO, 128, NCH, 512).transpose(2, 1, 0, 3)
            .reshape(NCH, 128, KO * 512)).astype(BF16_NP)
        m["ws1"] = ws1p
        m["ws3"] = ws3p
        m["ws2"] = ws2p
        m["bs1"] = bs1p
        m["bs3"] = bs3p
        in_maps.append(m)

    res = run_bass_kernel_spmd(nc, in_maps, list(range(N_CORES)))

    # host combine: scatter slot outputs, apply combine weights + b2, add
    # shared partials + bs2
    y = np.zeros((T, D), np.float32)
    for c in range(N_CORES):
        out = res.results[c]["oe"].astype(np.float32).reshape(D, C)
        for j, s in enumerate(profile):
            piece = assignment[c][j]
            if piece is not None:
                e, s0, n = piece
                tk = toks[e][s0:s0 + n]
                cwe = cw[tk, e][:, None]
                y[tk] += cwe * out[:, cbase[j]:cbase[j] + n].T
                y[tk] += cwe * b2[e][None, :]
        y[c * SH_TOK:(c + 1) * SH_TOK] += \
            out[:, cbase[-1]:cbase[-1] + SH_TOK].T
    y += bs2[None, :]
    return y.reshape(x.shape).astype(np.float32)


# revision 21
# speedup vs baseline: 1.0100x; 1.0029x over previous
"""Trainium2 Bass kernel for nn_MoE_32332513804634.

MoE: 16 routed experts (top-6, softmax-then-bias routing) + dense shared
expert, T=4096 tokens, D=2048, H=1408, HS=2816, fp32.

Strategy (8 NeuronCores, SPMD):
  - Host computes the gate (cheap: 0.27 GFLOP) and per-expert token lists.
  - Uniform slot profile: a small search finds per-core slot sizes
    (multiples of 128, identical across cores -- SPMD needs one program)
    that cover the 16 expert token counts with minimal padding (~4% vs
    the 17% of fixed 2048/512 caps). Each slot is bound per-core to one
    (expert, offset, len) piece; every core runs exactly the same
    instruction stream over sum(profile) routed tokens.
  - Within a slot: x is resident in SBUF, weights stream hm-outer so each
    weight byte is DMA'd once per slot. Layer-2 output is produced
    transposed [D, tokens] (tokens stay on the moving dim -- cost scales
    exactly with tokens, no 128-alignment of token counts needed).
  - Shared expert is token-parallel: each core runs its 512-token slice
    through the full 2816 hidden dim as one extra slot (no padding).
  - bf16 matmul operands + bf16 outputs; fp32 PSUM accumulation. Combine
    weights, b2/bs2 biases, scatter and the 8-way shared sum happen on
    the host in fp32.
"""

import sys
import numpy as np

sys.path.insert(0, "/opt/trn_rl_repo")

import concourse.bass as bass  # noqa: E402
import concourse.tile as tile  # noqa: E402
from concourse import bacc, mybir  # noqa: E402
from concourse.bass_utils import run_bass_kernel_spmd  # noqa: E402

T = 4096
D = 2048
H = 1408
E = 16
TOP_K = 6
HS = 2816
N_CORES = 8
KO = D // 128           # 16
HM = H // 128           # 11
HMS = HS // 128         # 22
SH_TOK = T // N_CORES   # 512 shared-expert tokens per core
MAX_SLOT = 1024
F32 = mybir.dt.float32
BF16 = mybir.dt.bfloat16
MM_DT = BF16

_PROGRAM_CACHE: dict = {}

import ml_dtypes  # noqa: E402

BF16_NP = ml_dtypes.bfloat16


def _host_gate(xf, gate_w, gate_b):
    """Numpy replica of the reference gate. Returns cw [T, E] dense combine
    weights and per-expert token lists (ascending)."""
    scores = xf @ gate_w.T
    m = scores.max(axis=-1, keepdims=True)
    p = np.exp(scores - m, dtype=np.float32)
    probs = p / p.sum(axis=-1, keepdims=True)
    biased = probs + gate_b
    idx = np.argpartition(biased, E - TOP_K, axis=1)[:, E - TOP_K:]
    mask = np.zeros((xf.shape[0], E), dtype=bool)
    mask[np.arange(xf.shape[0])[:, None], idx] = True
    cw = np.where(mask, probs, 0.0).astype(np.float32)
    toks = [np.flatnonzero(mask[:, e]).astype(np.int64) for e in range(E)]
    return cw, toks


def _cover(cs, slots):
    """Greedy cover: for each count (desc) take largest remaining slots until
    covered, then shrink the last taken to the smallest adequate one."""
    if not cs:
        return []
    c = cs[0][1]
    take = []
    s = 0
    for sz in slots:
        if s >= c:
            break
        take.append(sz)
        s += sz
    if s < c:
        return None
    if take:
        need = c - (s - take[-1])
        rest = list(slots)
        for t in take[:-1]:
            rest.remove(t)
        cands = [sz for sz in set(rest) if sz >= need]
        if cands:
            take = take[:-1] + [min(cands)]
    rem = list(slots)
    for t in take:
        rem.remove(t)
    rem.sort(reverse=True)
    sub = _cover(cs[1:], rem)
    if sub is None:
        return None
    return [take] + sub


def _plan_profile(counts):
    """Find per-core slot sizes (desc, multiples of 128, <= MAX_SLOT) common
    to all cores that cover the expert counts with minimal total padding.

    Returns (profile, assignment) where assignment[core][j] is
    (expert, start, n) or None."""
    order = np.argsort(counts)[::-1]
    cs = [(int(e), int(counts[e])) for e in order if counts[e] > 0]

    def parts(n, maxp, maxsz):
        if n == 0:
            yield ()
            return
        if maxp == 0:
            return
        for sz in range(min(n, maxsz), 0, -1):
            for rest in parts(n - sz, maxp - 1, sz):
                yield (sz,) + rest

    found = None
    for total_u in range(-(-sum(counts) // (128 * N_CORES)), 64):
        for prof_u in parts(total_u, 7, MAX_SLOT // 128):
            prof = tuple(s * 128 for s in prof_u)
            slots = sorted(list(prof) * N_CORES, reverse=True)
            r = _cover(cs, slots)
            if r is not None:
                found = (prof, r)
                break
        if found:
            break
    assert found is not None
    prof, groups = found
    # order slots big-then-its-small: each weight-DMA-heavy small slot runs
    # right after a big slot whose compute window prefetches its weights
    ps = sorted(prof, reverse=True)
    inter = []
    for i in range((len(ps) + 1) // 2):
        inter.append(ps[i])
        j = len(ps) - 1 - i
        if j > i:
            inter.append(ps[j])
    prof = tuple(inter)
    # slot instances: per size, list of (core, j)
    inst = {}
    for j, sz in enumerate(prof):
        for c in range(N_CORES):
            inst.setdefault(sz, []).append((c, j))
    assignment = [[None] * len(prof) for _ in range(N_CORES)]
    for (e, cnt), gslots in zip(cs, groups):
        off = 0
        for sz in sorted(gslots, reverse=True):
            c, j = inst[sz].pop()
            n = min(sz, cnt - off)
            if n > 0:
                assignment[c][j] = (e, off, n)
            off += n
    return prof, assignment


def _build_program(profile):
    """SPMD Bass program: len(profile) routed slots + 1 shared slot."""
    nc = bacc.Bacc("TRN2", debug=False, num_devices=N_CORES)

    sizes = list(profile) + [SH_TOK]
    nch = [-(-s // 512) for s in sizes]    # chunks per slot
    NCH = sum(nch)                          # total 512-wide chunk blocks
    C = NCH * 512                           # col space (chunk grid)

    ins = {}

    def din(name, shape, dt=MM_DT):
        ins[name] = nc.dram_tensor(name, list(shape), dt, kind="ExternalInput").ap()
        return ins[name]

    # x gathered tokens, chunk-major: 16 KB contiguous per partition row
    xg = din("xg", (NCH, 128, KO * 512))
    for j, s in enumerate(profile):
        din(f"w1_{j}", (HM, 128, KO, 128))
        din(f"w3_{j}", (HM, 128, KO, 128))
        din(f"w2_{j}", (KO, 128, HM, 128))
        din(f"b1_{j}", (128, HM), F32)
        din(f"b3_{j}", (128, HM), F32)
    din("ws1", (HMS, 128, KO, 128))
    din("ws3", (HMS, 128, KO, 128))
    din("ws2", (KO, 128, HMS, 128))
    din("bs1", (128, HMS), F32)
    din("bs3", (128, HMS), F32)
    oe = nc.dram_tensor("oe", [KO, 128, C], BF16, kind="ExternalOutput").ap()

    h_flat = max(HM * max(profile), HMS * SH_TOK)

    with tile.TileContext(nc) as tc:
        with (
            tc.tile_pool(name="xpool", bufs=3) as xpool,
            tc.tile_pool(name="hpool", bufs=1) as hpool,
            tc.tile_pool(name="wcol", bufs=11) as wcol,
            tc.tile_pool(name="w2pool", bufs=4) as w2pool,
            tc.tile_pool(name="tmp", bufs=2) as tmp,
            tc.tile_pool(name="opool", bufs=8) as opool,
            tc.tile_pool(name="cpool", bufs=1) as cpool,
            tc.tile_pool(name="pp", bufs=2, space="PSUM") as pp,
        ):
            # warm the PE (and its HAM clock gate) with throwaway matmuls
            # while the first slot's x/weight DMAs are in flight
            wsc1 = cpool.tile([128, 128], MM_DT, tag="wsc1")
            wsc2 = cpool.tile([128, 512], MM_DT, tag="wsc2")
            nc.vector.memset(wsc1[:], 0)
            nc.vector.memset(wsc2[:], 0)
            for _ in range(40):
                pw = pp.tile([128, 512], F32, tag="ps1")
                nc.tensor.matmul(pw[:], wsc1[:], wsc2[:], start=True,
                                 stop=True)
            def mlp_slot(cb, s, n_hm, w1_ap, w3_ap, w2_ap, b1_ap, b3_ap, tag):
                """out chunk-blocks [cb, cb+nch) = swiglu_mlp(x blocks)."""
                chunks = [(t0, min(512, s - t0)) for t0 in range(0, s, 512)]
                ncs = len(chunks)

                b1sb = cpool.tile([128, n_hm], F32, tag=f"b1{tag}")
                b3sb = cpool.tile([128, n_hm], F32, tag=f"b3{tag}")
                nc.scalar.dma_start(b1sb[:], b1_ap)
                nc.scalar.dma_start(b3sb[:], b3_ap)

                xts = []
                for ci, (t0, n) in enumerate(chunks):
                    xt = xpool.tile([128, KO, 512], MM_DT, tag="x")
                    nc.scalar.dma_start(
                        xt[:], xg[cb + ci].rearrange("p (a b) -> p a b", a=KO))
                    xts.append(xt)

                hsb = hpool.tile([128, h_flat], MM_DT, tag="h")
                # w2 for the first dms: emitted up front so the scalar queue
                # fires them during L1 (their ring waits are already met)
                w2ts = []
                for dm in range(4):
                    w2t = w2pool.tile([128, HMS, 128], MM_DT, tag="w2c")
                    nc.scalar.dma_start(w2t[:, :n_hm, :], w2_ap[dm])
                    w2ts.append(w2t)
                # ---- layer 1: h = silu(x@W1+b1) * (x@W3+b3), hm-outer ----
                for hm in range(n_hm):
                    w1t = wcol.tile([128, KO, 128], MM_DT, tag="w1c")
                    nc.sync.dma_start(w1t[:], w1_ap[hm])
                    w3t = wcol.tile([128, KO, 128], MM_DT, tag="w3c")
                    nc.sync.dma_start(w3t[:], w3_ap[hm])
                    for ci, (t0, n) in enumerate(chunks):
                        xt = xts[ci]
                        ps1 = pp.tile([128, 512], F32, tag="ps1")
                        for ko in range(KO):
                            nc.tensor.matmul(ps1[:, :n], w1t[:, ko, :],
                                             xt[:, ko, :n],
                                             start=(ko == 0),
                                             stop=(ko == KO - 1))
                        ps3 = pp.tile([128, 512], F32, tag="ps3")
                        for ko in range(KO):
                            nc.tensor.matmul(ps3[:, :n], w3t[:, ko, :],
                                             xt[:, ko, :n],
                                             start=(ko == 0),
                                             stop=(ko == KO - 1))
                        t1 = tmp.tile([128, 512], BF16, tag="t1")
                        nc.scalar.activation(t1[:, :n], ps1[:, :n],
                                             mybir.ActivationFunctionType.Silu,
                                             bias=b1sb[:, hm:hm + 1])
                        t3 = tmp.tile([128, 512], BF16, tag="t3")
                        nc.vector.tensor_scalar_add(t3[:, :n], ps3[:, :n],
                                                    b3sb[:, hm:hm + 1])
                        ho = hm * s + t0
                        nc.vector.tensor_mul(hsb[:, ho:ho + n],
                                             t1[:, :n], t3[:, :n])
                # ---- layer 2: out[d, t] = sum_h W2[d, h] h[h, t] ----
                for dm in range(KO):
                    w2t = w2ts[dm]
                    ob = opool.tile([128, 1024], BF16, tag="ob")
                    for ci, (t0, n) in enumerate(chunks):
                        ps2 = pp.tile([128, 512], F32, tag="ps2", bufs=4)
                        for hk in range(n_hm):
                            ho = hk * s + t0
                            nc.tensor.matmul(ps2[:, :n], w2t[:, hk, :],
                                             hsb[:, ho:ho + n],
                                             start=(hk == 0),
                                             stop=(hk == n_hm - 1))
                        nc.vector.tensor_copy(ob[:, t0:t0 + n], ps2[:, :n])
                    # single store per dm: 2-chunk slots get 2 KB rows
                    nc.scalar.dma_start(
                        oe[dm][:, cb * 512:(cb + ncs) * 512],
                        ob[:, :ncs * 512])
                    if dm + 4 < KO:
                        w2t = w2pool.tile([128, HMS, 128], MM_DT, tag="w2c")
                        nc.scalar.dma_start(w2t[:, :n_hm, :], w2_ap[dm + 4])
                        w2ts.append(w2t)

            cb = 0
            for j, s in enumerate(profile):
                mlp_slot(cb, s, HM, ins[f"w1_{j}"], ins[f"w3_{j}"],
                         ins[f"w2_{j}"], ins[f"b1_{j}"], ins[f"b3_{j}"],
                         f"e{j}")
                cb += -(-s // 512)
            mlp_slot(cb, SH_TOK, HMS, ins["ws1"], ins["ws3"], ins["ws2"],
                     ins["bs1"], ins["bs3"], "sh")

    nc.compile()
    return nc


def _pack_w13(w):
    """[H', D] -> [H'/128, 128(d%128), KO, 128(h%128)] bf16 (lhsT tiles)."""
    hm = w.shape[0] // 128
    return np.ascontiguousarray(
        w.reshape(hm, 128, KO, 128).transpose(0, 3, 2, 1)).astype(BF16_NP)


def _pack_w2(w):
    """[D, H'] -> [KO, 128(h%128), H'/128, 128(d%128)] bf16 (lhsT tiles)."""
    hm = w.shape[1] // 128
    return np.ascontiguousarray(
        w.reshape(KO, 128, hm, 128).transpose(0, 3, 2, 1)).astype(BF16_NP)


def kernel(x, gate_w, gate_b, w1, b1, w2, b2, w3, b3,
           ws1, bs1, ws2, bs2, ws3, bs3):
    x = np.asarray(x, np.float32)
    xf = np.ascontiguousarray(x.reshape(-1, D))
    gate_w = np.asarray(gate_w, np.float32)
    gate_b = np.asarray(gate_b, np.float32)
    w1 = np.asarray(w1, np.float32)
    b1 = np.asarray(b1, np.float32)
    w2 = np.asarray(w2, np.float32)
    b2 = np.asarray(b2, np.float32)
    w3 = np.asarray(w3, np.float32)
    b3 = np.asarray(b3, np.float32)
    ws1 = np.asarray(ws1, np.float32)
    bs1 = np.asarray(bs1, np.float32)
    ws2 = np.asarray(ws2, np.float32)
    bs2 = np.asarray(bs2, np.float32)
    ws3 = np.asarray(ws3, np.float32)
    bs3 = np.asarray(bs3, np.float32)

    cw, toks = _host_gate(xf, gate_w, gate_b)
    counts = np.array([len(t) for t in toks])
    profile, assignment = _plan_profile(counts)

    if profile not in _PROGRAM_CACHE:
        _PROGRAM_CACHE[profile] = _build_program(profile)
    nc = _PROGRAM_CACHE[profile]

    # chunk-grid column bases: each slot occupies ceil(s/512) 512-wide blocks
    sizes = list(profile) + [SH_TOK]
    cbase = []
    acc = 0
    for s in sizes:
        cbase.append(acc * 512)
        acc += -(-s // 512)
    NCH = acc
    C = NCH * 512
    xT = xf.T  # [D, T] view

    # per-expert packed weights, shared across cores/slots
    need = sorted({p[0] for slots in assignment for p in slots if p is not None})
    w1p = {e: _pack_w13(w1[e]) for e in need}
    w3p = {e: _pack_w13(w3[e]) for e in need}
    w2p = {e: _pack_w2(w2[e]) for e in need}
    b1p = {e: np.ascontiguousarray(b1[e].reshape(HM, 128).T) for e in need}
    b3p = {e: np.ascontiguousarray(b3[e].reshape(HM, 128).T) for e in need}
    zb = np.zeros((128, HM), np.float32)

    ws1p = _pack_w13(ws1)
    ws3p = _pack_w13(ws3)
    ws2p = _pack_w2(ws2)
    bs1p = np.ascontiguousarray(bs1.reshape(HMS, 128).T)
    bs3p = np.ascontiguousarray(bs3.reshape(HMS, 128).T)

    in_maps = []
    for c in range(N_CORES):
        m = {}
        xcols = np.zeros((D, C), np.float32)
        for j, s in enumerate(profile):
            piece = assignment[c][j]
            if piece is None:
                e0 = need[0]
                m[f"w1_{j}"] = w1p[e0]
                m[f"w3_{j}"] = w3p[e0]
                m[f"w2_{j}"] = w2p[e0]
                m[f"b1_{j}"] = zb
                m[f"b3_{j}"] = zb
            else:
                e, s0, n = piece
                tk = toks[e][s0:s0 + n]
                xcols[:, cbase[j]:cbase[j] + n] = xT[:, tk]
                m[f"w1_{j}"] = w1p[e]
                m[f"w3_{j}"] = w3p[e]
                m[f"w2_{j}"] = w2p[e]
                m[f"b1_{j}"] = b1p[e]
                m[f"b3_{j}"] = b3p[e]
        xcols[:, cbase[-1]:cbase[-1] + SH_TOK] = \
            xT[:, c * SH_TOK:(c + 1) * SH_TOK]
        m["xg"] = np.ascontiguousarray(
            xcols.reshape(KO, 128, NCH, 512).transpose(2, 1, 0, 3)
            .reshape(NCH, 128, KO * 512)).astype(BF16_NP)
        m["ws1"] = ws1p
        m["ws3"] = ws3p
        m["ws2"] = ws2p
        m["bs1"] = bs1p
        m["bs3"] = bs3p
        in_maps.append(m)

    res = run_bass_kernel_spmd(nc, in_maps, list(range(N_CORES)))

    # host combine: scatter slot outputs, apply combine weights + b2, add
    # shared partials + bs2
    y = np.zeros((T, D), np.float32)
    for c in range(N_CORES):
        out = res.results[c]["oe"].astype(np.float32).reshape(D, C)
        for j, s in enumerate(profile):
            piece = assignment[c][j]
            if piece is not None:
                e, s0, n = piece
                tk = toks[e][s0:s0 + n]
                cwe = cw[tk, e][:, None]
                y[tk] += cwe * out[:, cbase[j]:cbase[j] + n].T
                y[tk] += cwe * b2[e][None, :]
        y[c * SH_TOK:(c + 1) * SH_TOK] += \
            out[:, cbase[-1]:cbase[-1] + SH_TOK].T
    y += bs2[None, :]
    return y.reshape(x.shape).astype(np.float32)
